# revision 16
# baseline (speedup 1.0000x reference)
"""Trainium2 Bass kernel: fused multi-head causal self-attention block.

Computes, for x:(B,S,H), W_qkv:(3H,H), b_qkv:(3H,), W_out:(H,H), b_out:(H,):
    qkv = x @ W_qkv.T + b_qkv ; split into q,k,v heads (NH heads, D=H/NH)
    out = softmax(causal(q k^T / sqrt(D))) v   ; merge heads
    return out @ W_out.T + b_out

Sharding over 8 NeuronCores: DP(2 batches) x TP(4 head-groups).
Core c handles batch b=c//4, head group g=c%4 (heads 4g..4g+3).
After per-head attention, the per-head outputs (stored transposed, [D,S])
are AllGather'd within each batch group of 4 cores; each core then computes
a disjoint 512-column slice of the output projection, so the host does a
pure concatenation (no host-side arithmetic beyond layout).

All device matmuls run as float32r (full-rate PE path) by default; storage
and accumulation are fp32.
"""

import math

import numpy as np

import concourse.bass as bass
import concourse.mybir as mybir
import concourse.tile as tile
from concourse import bacc
from concourse.bass_utils import run_bass_kernel_spmd

FP = mybir.dt.float32
FR = mybir.dt.float32r

# Full-size problem constants.
B, S, H, NH = 2, 2048, 2048, 16
D = 128
NCORES = 8
GROUPS = 4                  # head-groups per batch (TP degree)
REPLICA_GROUPS = [[0, 1, 2, 3], [4, 5, 6, 7]]

USE_F32R = True             # float32r matmuls (1 cyc/row) vs float32 (4 cyc/row)
MM_DT = FR if USE_F32R else FP
TRACE = False               # set by test harness to capture NTFF profile
LAST_EXEC_NS = None
LAST_RESULTS = None


def build_nc(s=S, h=H, nh=NH, reps=1):
    """Build the SPMD Bass program (identical on all 8 cores).

    reps>1 repeats the whole computation in one NEFF; used only by the
    timing harness ((T(K)-T(1))/(K-1) cancels the dispatch overhead).
    """
    nl = nh // GROUPS           # local heads per core
    dg = nl * D                 # per-core slice of the head dim
    hc = h // 128               # contraction chunks for the projections
    sq = s // 512               # 512-wide q strips
    st_n = s // 128             # 128-row s tiles
    scale = 1.0 / math.sqrt(D)

    nc = bacc.Bacc(
        "TRN2",
        target_bir_lowering=False,
        debug=False,
        enable_asserts=False,
        num_devices=NCORES,
    )

    # ---- I/O -----------------------------------------------------------
    xT_d = nc.dram_tensor("xT", [h, s], MM_DT, kind="ExternalInput")
    wq_d = nc.dram_tensor("wq", [h, dg], MM_DT, kind="ExternalInput")
    wk_d = nc.dram_tensor("wk", [h, dg], MM_DT, kind="ExternalInput")
    wv_d = nc.dram_tensor("wv", [h, dg], MM_DT, kind="ExternalInput")
    wo_d = nc.dram_tensor("wo", [h, dg], MM_DT, kind="ExternalInput")
    bq_d = nc.dram_tensor("bq", [128, nl], FP, kind="ExternalInput")
    bk_d = nc.dram_tensor("bk", [128, nl], FP, kind="ExternalInput")
    bv_d = nc.dram_tensor("bv", [128, dg], FP, kind="ExternalInput")
    bo_d = nc.dram_tensor("bo", [128, dg], FP, kind="ExternalInput")
    mask_d = nc.dram_tensor("mask", [128, 896], FP, kind="ExternalInput")
    ones_d = nc.dram_tensor("ones", [128, 128], MM_DT, kind="ExternalInput")
    out_d = nc.dram_tensor("out", [s, dg], FP, kind="ExternalOutput")

    with tile.TileContext(nc) as tc:
        with tc.tile_pool(name="const", bufs=1) as constp:
            mask_sb = constp.tile([128, 896], FP)
            nc.sync.dma_start(mask_sb[:], mask_d[:])
            ones_sb = constp.tile([128, 128], MM_DT)
            onesf_sb = constp.tile([1, 128], FP)
            nc.vector.memset(onesf_sb[:], 1.0)
            nc.sync.dma_start(ones_sb[:], ones_d[:])
            bq_sb = constp.tile([128, nl], FP)
            nc.sync.dma_start(bq_sb[:], bq_d[:])
            bk_sb = constp.tile([128, nl], FP)
            nc.sync.dma_start(bk_sb[:], bk_d[:])
            bv_sb = constp.tile([128, dg], FP)
            nc.sync.dma_start(bv_sb[:], bv_d[:])
            bo_sb = constp.tile([128, dg], FP)
            nc.sync.dma_start(bo_sb[:], bo_d[:])
            ones_col = ones_sb[:, 0:1]        # [128,1] lhsT for denominator sum
            ones_row = onesf_sb[0:1, :]       # [1,128] fp32 lhsT for partition-broadcast

            for _rep in range(reps):
                _emit_body(nc, tc, s, h, nh,
                           xT_d, wq_d, wk_d, wv_d, wo_d, out_d,
                           bq_sb, bk_sb, bv_sb, bo_sb,
                           mask_sb, ones_col, ones_row, scale)

    nc.compile()
    return nc


def _emit_body(nc, tc, s, h, nh,
               xT_d, wq_d, wk_d, wv_d, wo_d, out_d,
               bq_sb, bk_sb, bv_sb, bo_sb,
               mask_sb, ones_col, ones_row, scale):
    nl = nh // GROUPS
    dg = nl * D
    hc = h // 128
    sq = s // 512
    st_n = s // 128
    if True:
        if True:
            with tc.tile_pool(name="qkv", bufs=1) as qkvp:
                qT = [qkvp.tile([128, s], MM_DT, tag=f"qT{t}", name=f"qT{t}") for t in range(nl)]
                kT = [qkvp.tile([128, s], MM_DT, tag=f"kT{t}", name=f"kT{t}") for t in range(nl)]
                vv = [qkvp.tile([128, dg], MM_DT, tag=f"v{t}", name=f"v{t}") for t in range(st_n)]

                # ---- Phase A1: Q^T and K^T projections ------------------
                with tc.tile_pool(name="wqk", bufs=1) as wqkp, \
                     tc.tile_pool(name="xA", bufs=5) as xap, \
                     tc.tile_pool(name="psA", bufs=1, space="PSUM") as psA:
                    wq_sb = [wqkp.tile([128, dg], MM_DT, tag=f"wq{hh}", name=f"wq{hh}") for hh in range(hc)]
                    wk_sb = [wqkp.tile([128, dg], MM_DT, tag=f"wk{hh}", name=f"wk{hh}") for hh in range(hc)]
                    for hh in range(hc):
                        nc.sync.dma_start(wq_sb[hh][:], wq_d[128 * hh:128 * hh + 128, :])
                        nc.sync.dma_start(wk_sb[hh][:], wk_d[128 * hh:128 * hh + 128, :])
                    for strip in range(sq):
                        cs = slice(512 * strip, 512 * strip + 512)
                        pss = [psA.tile([128, 512], FP, tag=f"psqk{gi}", name=f"psqk{gi}")
                               for gi in range(2 * nl)]
                        for hh in range(hc):
                            xch = xap.tile([128, 512], MM_DT, tag="xch", name="xch")
                            nc.sync.dma_start(xch[:], xT_d[128 * hh:128 * hh + 128, cs])
                            for gi in range(2 * nl):
                                w_sb = wq_sb if gi < nl else wk_sb
                                t = gi % nl
                                nc.tensor.matmul(
                                    pss[gi][:],
                                    w_sb[hh][:, 128 * t:128 * t + 128],
                                    xch[:],
                                    start=(hh == 0), stop=(hh == hc - 1),
                                )
                        for gi in range(2 * nl):
                            dstT = qT if gi < nl else kT
                            bias = bq_sb if gi < nl else bk_sb
                            t = gi % nl
                            nc.scalar.activation(
                                dstT[t][:, cs], pss[gi][:],
                                mybir.ActivationFunctionType.Identity,
                                bias=bias[:, t:t + 1],
                            )

                # ---- Phase A2: V projection (natural [s, d] layout) -----
                with tc.tile_pool(name="wvp", bufs=1) as wvp, \
                     tc.tile_pool(name="xV", bufs=5) as xvp, \
                     tc.tile_pool(name="psV", bufs=2, space="PSUM") as psV:
                    wv_sb = [wvp.tile([128, dg], MM_DT, tag=f"wv{hh}", name=f"wv{hh}") for hh in range(hc)]
                    for hh in range(hc):
                        nc.sync.dma_start(wv_sb[hh][:], wv_d[128 * hh:128 * hh + 128, :])
                    for strip in range(sq):
                        cs = slice(512 * strip, 512 * strip + 512)
                        psv = [psV.tile([128, dg], FP, tag=f"psv{sti}", name=f"psv{sti}")
                               for sti in range(4)]
                        for hh in range(hc):
                            xch2 = xvp.tile([128, 512], MM_DT, tag="xch2", name="xch2")
                            nc.sync.dma_start(xch2[:], xT_d[128 * hh:128 * hh + 128, cs])
                            for sti in range(4):
                                nc.tensor.matmul(
                                    psv[sti][:],
                                    xch2[:, 128 * sti:128 * sti + 128],
                                    wv_sb[hh][:],
                                    start=(hh == 0), stop=(hh == hc - 1),
                                )
                        for sti in range(4):
                            nc.vector.tensor_add(vv[4 * strip + sti][:], psv[sti][:], bv_sb[:])

                # ---- Phase B: attention per local head + AllGather ------
                with tc.tile_pool(name="wop", bufs=1) as wop, \
                     tc.tile_pool(name="etp", bufs=6) as etp, \
                     tc.tile_pool(name="atp", bufs=2) as atp, \
                     tc.tile_pool(name="rbp", bufs=2) as rbp, \
                     tc.tile_pool(name="dramp", bufs=1, space="DRAM") as dramp:

                    wo_sb = [wop.tile([128, dg], MM_DT, tag=f"wo{ci}", name=f"wo{ci}") for ci in range(hc)]
                    for ci in range(hc):
                        nc.sync.dma_start(wo_sb[ci][:], wo_d[128 * ci:128 * ci + 128, :])

                    agouts = []
                    psS_cm = tc.tile_pool(name="psS", bufs=3, space="PSUM")
                    psAV_cm = tc.tile_pool(name="psAV", bufs=2, space="PSUM")
                    psDN_cm = tc.tile_pool(name="psDN", bufs=2, space="PSUM")
                    psS, psAV, psDN = psS_cm.__enter__(), psAV_cm.__enter__(), psDN_cm.__enter__()
                    for l in range(nl):
                        aT = atp.tile([128, s], MM_DT, tag="aT", name="aT")
                        for qs in range(sq):
                            qsl = slice(512 * qs, 512 * qs + 512)
                            ps_av = psAV.tile([128, 512], FP, tag="ps_av", name="ps_av")
                            ps_dn = psDN.tile([1, 512], FP, tag="ps_dn", name="ps_dn")
                            nk = 4 * qs + 4
                            for kt in range(nk):
                                ps_s = psS.tile([128, 512], FP, tag="ps_s", name="ps_s")
                                nc.tensor.matmul(
                                    ps_s[:],
                                    kT[l][:, 128 * kt:128 * kt + 128],
                                    qT[l][:, qsl],
                                    start=True, stop=True,
                                )
                                et = etp.tile([128, 512], MM_DT, tag="et", name="et")
                                nc.scalar.activation(
                                    et[:], ps_s[:],
                                    mybir.ActivationFunctionType.Exp,
                                    scale=scale,
                                )
                                off = 128 * kt - 512 * qs
                                if off >= 0:  # partial (diagonal) tile: apply causal mask
                                    nc.vector.tensor_mul(
                                        et[:], et[:], mask_sb[:, 384 - off:896 - off])
                                nc.tensor.matmul(
                                    ps_dn[:], ones_col, et[:],
                                    start=(kt == 0), stop=(kt == nk - 1),
                                )
                                nc.tensor.matmul(
                                    ps_av[:],
                                    vv[kt][:, 128 * l:128 * l + 128],
                                    et[:],
                                    start=(kt == 0), stop=(kt == nk - 1),
                                )
                            # normalize: aT[:, qsl] = ps_av * (1/denom) broadcast
                            dn_sb = rbp.tile([1, 512], FP, tag="dn_sb", name="dn_sb")
                            nc.vector.tensor_copy(dn_sb[:], ps_dn[:])
                            ps_rb = psDN.tile([128, 512], FP, tag="ps_rb", name="ps_rb",
                                              bufs=1)
                            nc.tensor.matmul(
                                ps_rb[:], ones_row, dn_sb[:],
                                start=True, stop=True,
                            )
                            rb_sb = rbp.tile([128, 512], FP, tag="rb_sb", name="rb_sb")
                            nc.vector.reciprocal(rb_sb[:], ps_rb[:])
                            nc.vector.tensor_mul(aT[:, qsl], ps_av[:], rb_sb[:])
                        # ship this head's A^T and AllGather across the batch group
                        agin = dramp.tile([128, s], MM_DT, tag=f"agin{l}", name=f"agin{l}")
                        nc.sync.dma_start(agin[:], aT[:])
                        agout = dramp.tile([512, s], MM_DT, tag=f"agout{l}", name=f"agout{l}")
                        nc.gpsimd.collective_compute(
                            "AllGather",
                            mybir.AluOpType.bypass,
                            replica_groups=REPLICA_GROUPS,
                            ins=[agin.opt()],
                            outs=[agout.opt()],
                        )
                        agouts.append(agout)

                    # release attention PSUM pools before phase C allocates
                    psDN_cm.__exit__(None, None, None)
                    psAV_cm.__exit__(None, None, None)
                    psS_cm.__exit__(None, None, None)

                    # ---- Phase C: output projection (512-col slice) -----
                    with tc.tile_pool(name="atsp", bufs=8) as atsp, \
                         tc.tile_pool(name="outp", bufs=2) as outp, \
                         tc.tile_pool(name="psO", bufs=2, space="PSUM") as psO:
                        for sti in range(st_n):
                            rs = slice(128 * sti, 128 * sti + 128)
                            ps_o = psO.tile([128, dg], FP, tag="ps_o", name="ps_o")
                            ci = 0
                            for l in range(nl):
                                for r in range(4):
                                    atile = atsp.tile([128, 128], MM_DT, tag="atile", name="atile")
                                    nc.sync.dma_start(
                                        atile[:], agouts[l][128 * r:128 * r + 128, rs])
                                    nc.tensor.matmul(
                                        ps_o[:], atile[:], wo_sb[ci][:],
                                        start=(ci == 0), stop=(ci == hc - 1),
                                    )
                                    ci += 1
                            ob = outp.tile([128, dg], FP, tag="ob", name="ob")
                            nc.vector.tensor_add(ob[:], ps_o[:], bo_sb[:])
                            nc.sync.dma_start(out_d[rs, :], ob[:])


def make_inputs(x, W_qkv, b_qkv, W_out, b_out, s=S, h=H, nh=NH):
    """Host-side sharding: per-core input dicts."""
    nl = nh // GROUPS
    dg = nl * D
    x = np.ascontiguousarray(np.asarray(x, dtype=np.float32))
    W_qkv = np.asarray(W_qkv, dtype=np.float32)
    b_qkv = np.asarray(b_qkv, dtype=np.float32)
    W_out = np.asarray(W_out, dtype=np.float32)
    b_out = np.asarray(b_out, dtype=np.float32)

    # causal staircase master mask: mask[i, u] = 1 iff u >= i + 384
    uu = np.arange(896)[None, :]
    ii = np.arange(128)[:, None]
    mask = (uu >= ii + 384).astype(np.float32)
    ones = np.ones((128, 128), dtype=np.float32)

    WoT = W_out.T  # [h (d-in), h (n-out)]
    in_maps = []
    for c in range(NCORES):
        b, g = divmod(c, GROUPS)
        xT = np.ascontiguousarray(x[b].T)                       # [h, s]
        wq = np.ascontiguousarray(W_qkv[dg * g:dg * (g + 1), :].T)
        wk = np.ascontiguousarray(W_qkv[h + dg * g:h + dg * (g + 1), :].T)
        wv = np.ascontiguousarray(W_qkv[2 * h + dg * g:2 * h + dg * (g + 1), :].T)
        bq = np.ascontiguousarray(
            b_qkv[dg * g:dg * (g + 1)].reshape(nl, 128).T)      # [128, nl]
        bk = np.ascontiguousarray(
            b_qkv[h + dg * g:h + dg * (g + 1)].reshape(nl, 128).T)
        bv = np.tile(b_qkv[2 * h + dg * g:2 * h + dg * (g + 1)][None, :], (128, 1))
        bo = np.tile(b_out[dg * g:dg * (g + 1)][None, :], (128, 1))
        # W_out^T rows permuted to the AllGather d-order:
        # ci = l*4 + r  ->  global head 4r + l (within this batch group)
        blocks = []
        for l in range(nl):
            for r in range(GROUPS):
                hh = nl * r + l  # head held as local-head l by group-rank r
                blocks.append(WoT[D * hh:D * (hh + 1), dg * g:dg * (g + 1)])
        wo = np.ascontiguousarray(np.concatenate(blocks, axis=0))  # [h, dg]
        in_maps.append({
            "xT": xT, "wq": wq, "wk": wk, "wv": wv, "wo": wo,
            "bq": bq, "bk": bk,
            "bv": np.ascontiguousarray(bv), "bo": np.ascontiguousarray(bo),
            "mask": mask, "ones": ones,
        })
    return in_maps


_NC_CACHE = {}


def _get_nc(key=(S, H, NH)):
    if key not in _NC_CACHE:
        _NC_CACHE[key] = build_nc(*key)
    return _NC_CACHE[key]


def kernel(x, W_qkv, b_qkv, W_out, b_out):
    global LAST_EXEC_NS, LAST_RESULTS
    nc = _get_nc()
    in_maps = make_inputs(x, W_qkv, b_qkv, W_out, b_out)
    res = run_bass_kernel_spmd(
        nc, in_maps, core_ids=list(range(NCORES)), trace=TRACE)
    LAST_EXEC_NS = res.exec_time_ns
    LAST_RESULTS = res
    nl = NH // GROUPS
    dg = nl * D
    out = np.empty((B, S, H), dtype=np.float32)
    for c in range(NCORES):
        b, g = divmod(c, GROUPS)
        out[b, :, dg * g:dg * (g + 1)] = res.results[c]["out"]
    return out
